# revision 1
# baseline (speedup 1.0000x reference)
"""Trainium2 Bass kernel for nn_Decoder_58514634440787 (histogram_binning).

Piecewise-linear decoder: y[b, s] = interp of (segment_x, segment_y) knots
evaluated at the uniform pixel grid t_s = (s+1)/S, S = 196608, B = 8.

Pixels are sharded across 8 cores (24576 each), laid out on-chip as
[128 partitions = 8 batches x 16 rows, 1536 pixels].  The pixel axis is
processed in column chunks (512/512/384/128).  For each chunk the host
builds a tiny piecewise-linear *basis* so the whole computation is one
single-pass fp16 matmul per chunk on the otherwise-idle TensorEngine:

    out[p, f] = a[p]*f + b[p] + sum_j w_j[p] * D_j[f]
    out_psum[128, C] = lhsT[R, 128].T @ M[R, C]

(a, b) is the line of the segment active at the chunk start for
partition p (rebased to chunk-local f, float64 host math), and each
basis row D_j covers one segment transition at chunk-local column k_j:
D_j[f] = (f >= k_j) ? (dalpha*f + dbeta) : 0 is the difference between
new and old segment lines, with w_j one-hot on the owning partition.
Summed left to right the deltas telescope, so every integer f gets
exactly the active segment's line -- jumps from zero-width segments
included, no continuity assumption.

fp16 keeps the matmul single-pass (fp32 needs two LOW/HIGH passes and
was 4x slower end-to-end on the PE).  Accuracy is preserved by:
  - the iota row is f * 2^-e with integer f -- exact in fp16;
  - every value row is split into hi/lo fp16 rows (hi = fp16(v),
    lo = fp16(v - hi)), recovering ~22-bit precision;
  - per-row power-of-2 scaling (folded into the paired lhsT entry,
    powers of two are exact) keeps magnitudes out of the subnormal
    range where fp16 rounding error would blow up.

Each chunk's [R, 128+C] fp16 block arrives as one small DMA (~10-20 KB)
split over the two HWDGE queues; PSUM->SBUF copies alternate between
the Vector and Scalar engines; stores alternate between the GpSimd
(SWDGE) and Sync (HWDGE) queues so no engine issues more than two DMA
triggers.  R adapts to the data and is bucketed for compile caching; a
chunk whose transition count would overflow the 128-partition
contraction limit is split column-wise, so any input fits.

Inputs are the full [8, 33] knot tensors; sharding/gather happens here.
"""

import numpy as np

S = 196608
B = 8
W = 1536              # pixels per partition row
RPB = 16              # rows per batch per core
P = 128               # partitions = B * RPB
NCORES = 8
PIX_PER_CORE = RPB * W  # 24576

_t_grid = None          # f32 [S] exact (s+1)/S
_compiled = {}          # layout tuple -> nc

_R_BUCKETS = (8, 12, 16, 24, 32, 48, 64, 96, 128)
_BASE_WIDTHS = (256, 512, 512, 256)


def _get_grid():
    global _t_grid
    if _t_grid is None:
        _t_grid = (np.arange(1, S + 1, dtype=np.float64) / S).astype(np.float32)
    return _t_grid


def _fix_x_order(sx, sy):
    """Running max of x along the segment axis, y carried from the position
    achieving the max (ties keep the later entry). Matches reference."""
    x = sx.copy()
    y = sy.copy()
    for b in range(sx.shape[0]):
        cx, cy = sx[b, 0], sy[b, 0]
        for i in range(sx.shape[1]):
            if sx[b, i] >= cx:
                cx, cy = sx[b, i], sy[b, i]
            x[b, i] = cx
            y[b, i] = cy
    return x, y


def _pow2_scale(vmax):
    """Power-of-two s with vmax*s ~ 2^4, so fp16(v*s) avoids subnormals
    and overflow.  The inverse 2^-e (|e| <= 24) is exact in fp16, and so
    is f * 2^-e for integer f <= 2047 (subnormals included)."""
    if vmax <= 0.0 or not np.isfinite(vmax):
        return 1.0
    e = 4 - int(np.floor(np.log2(vmax)))
    e = min(max(e, -10), 24)
    return float(2.0 ** e)


def _host_prep(segment_x, segment_y):
    """Returns (in_maps_arrays, layout).

    layout: tuple of (chunk_width, R) pairs, same for every core.
    in_maps_arrays: [core][chunk] -> fp16 [R, 128 + chunk_width] array
    holding lhsT (cols 0:128) and the moving basis M (cols 128:).
    """
    t_grid = _get_grid()
    sx = np.asarray(segment_x, dtype=np.float32)
    sy = np.asarray(segment_y, dtype=np.float32)
    x, y = _fix_x_order(sx, sy)

    gaps = x[:, 1:] - x[:, :-1]
    div = np.where(gaps == 0.0, np.float32(0.0001), gaps).astype(np.float32)
    a = ((y[:, 1:] - y[:, :-1]) / div).astype(np.float32)          # [B, 32]
    a64 = a.astype(np.float64)
    x64 = x.astype(np.float64)
    y64 = y.astype(np.float64)

    # First pixel index s with t_s >= x_n, for knots n = 1..31.
    # searchsorted on the exact f32 grid == the reference's f32 compares.
    k = np.stack([np.searchsorted(t_grid, x[b, 1:32], side='left')
                  for b in range(B)])                               # [B, 31]
    ks = [np.sort(k[b]) for b in range(B)]

    def seg(b, s):
        # segment index at pixel s = number of transition pixels <= s
        return int(np.searchsorted(ks[b], s, side='right'))

    def line64(b, m, s0):
        # (slope, intercept) in local column space for segment m of batch b,
        # f local to a span starting at global pixel s0, in float64
        aa = a64[b, m]
        return (aa / S, aa * ((s0 + 1) / S - x64[b, m]) + y64[b, m])

    # transitions[b] = sorted unique global pixels where the segment changes
    transitions = [np.unique(k[b][k[b] < S]) for b in range(B)]

    def chunk_transitions(widths):
        """[core][chunk] -> list of (p, k_loc, b, s0) transition entries.
        k_loc in [1, width): a transition at the chunk start is folded
        into the base line."""
        offs = np.concatenate([[0], np.cumsum(widths)]).astype(int)
        out = [[[] for _ in widths] for _ in range(NCORES)]
        for c in range(NCORES):
            for b in range(B):
                for r in range(RPB):
                    row0 = c * PIX_PER_CORE + r * W
                    p = b * RPB + r
                    tr = transitions[b]
                    lo = np.searchsorted(tr, row0, side='right')
                    hi = np.searchsorted(tr, row0 + W - 1, side='right')
                    for g in tr[lo:hi]:
                        col = int(g) - row0          # 1..W-1
                        ci = int(np.searchsorted(offs, col, side='right')) - 1
                        k_loc = col - int(offs[ci])
                        if k_loc == 0:
                            continue  # covered by that chunk's base line
                        out[c][ci].append((p, k_loc, b, row0 + int(offs[ci])))
        return out

    # choose chunk widths so every (core, chunk) fits 2*(2+n)<=128 rows
    widths = list(_BASE_WIDTHS)
    while True:
        per = chunk_transitions(widths)
        worst = [max(len(per[c][i]) for c in range(NCORES))
                 for i in range(len(widths))]
        bad = [i for i, n in enumerate(worst) if 2 * (2 + n) > 128]
        if not bad:
            break
        i = bad[0]
        w = widths[i]
        assert w >= 2, "cannot split further"
        widths = widths[:i] + [w // 2, w - w // 2] + widths[i + 1:]

    rs = []
    for i, n in enumerate(worst):
        need = 2 * (2 + n)
        rb = next(rr for rr in _R_BUCKETS if rr >= need)
        rs.append(rb)
    # chunks are shipped as two packed DMAs (first half / second half of
    # the chunk list); chunks sharing a DMA share the partition-dim R
    h = (len(rs) + 1) // 2
    ra = max(rs[:h]); rb = max(rs[h:])
    rs = [ra] * h + [rb] * (len(rs) - h)
    layout = tuple(zip(widths, rs))

    def hilo(v64):
        """Split float64 array/scalar into (hi, lo) fp16 pair."""
        hi = np.asarray(v64, dtype=np.float16)
        lo = np.asarray(v64 - hi.astype(np.float64), dtype=np.float16)
        return hi, lo

    offs = np.concatenate([[0], np.cumsum(widths)]).astype(int)
    arrays = []
    for c in range(NCORES):
        core_arrays = []
        for i, (cw, R) in enumerate(layout):
            arr = np.zeros((R, 128 + cw), dtype=np.float16)
            f = np.arange(cw, dtype=np.float64)

            # base lines per partition, rebased to this chunk's start
            av = np.zeros(P, dtype=np.float64)
            bv = np.zeros(P, dtype=np.float64)
            for b in range(B):
                for r in range(RPB):
                    p = b * RPB + r
                    s0 = c * PIX_PER_CORE + r * W + int(offs[i])
                    al, be = line64(b, seg(b, s0), s0)
                    av[p] = al
                    bv[p] = be

            # rows 0-1: a_hi/a_lo (in lhsT) paired with scaled iota (in M)
            sa = _pow2_scale(float(np.max(np.abs(av))))
            iota = (f / sa).astype(np.float16)  # f * 2^-e: exact
            ahi, alo = hilo(av * sa)
            arr[0, :128] = ahi
            arr[1, :128] = alo
            arr[0, 128:] = iota
            arr[1, 128:] = iota

            # rows 2-3: b_hi/b_lo paired with a scaled constant row
            sb = _pow2_scale(float(np.max(np.abs(bv))))
            bhi, blo = hilo(bv * sb)
            arr[2, :128] = bhi
            arr[3, :128] = blo
            arr[2, 128:] = np.float16(1.0 / sb)
            arr[3, 128:] = np.float16(1.0 / sb)

            # two rows per transition: D_hi / D_lo, one-hot scaled lhsT
            for j, (p, k_loc, b, s0) in enumerate(per[c][i]):
                m_new = seg(b, s0 + k_loc)
                m_old = seg(b, s0 + k_loc - 1)
                al_n, be_n = line64(b, m_new, s0)
                al_o, be_o = line64(b, m_old, s0)
                d = np.where(f >= k_loc,
                             (al_n - al_o) * f + (be_n - be_o), 0.0)
                sd = _pow2_scale(float(np.max(np.abs(d))))
                dhi, dlo = hilo(d * sd)
                arr[4 + 2 * j, 128:] = dhi
                arr[5 + 2 * j, 128:] = dlo
                arr[4 + 2 * j, p] = np.float16(1.0 / sd)
                arr[5 + 2 * j, p] = np.float16(1.0 / sd)
            core_arrays.append(arr)
        arrays.append(core_arrays)
    return arrays, layout


def _build(layout):
    import concourse.bacc as bacc
    import concourse.mybir as mybir
    from concourse.tile import TileContext

    f16 = mybir.dt.float16
    f32 = mybir.dt.float32

    nc = bacc.Bacc("TRN2", debug=False, enable_asserts=False,
                   enable_partition_id=False, monotonic_sem_count=0)
    h = (len(layout) + 1) // 2
    groups = [list(range(0, h)), list(range(h, len(layout)))]
    gw = [sum(128 + layout[i][0] for i in g) for g in groups]
    gr = [layout[g[0]][1] for g in groups]
    ins = [nc.dram_tensor(f"pm{gi}", [gr[gi], gw[gi]], f16,
                          kind="ExternalInput").ap()
           for gi in range(2)]
    y_dram = nc.dram_tensor("y", [P, W], f32, kind="ExternalOutput").ap()

    with TileContext(nc) as tc:
        with tc.tile_pool(name="pool", bufs=1) as pool, \
             tc.tile_pool(name="psum", bufs=1, space="PSUM") as psum_pool:
            # one packed DMA per chunk group: fewer HWDGE trigger
            # instructions (they cost ~1us of engine time each)
            gtiles = []
            for gi in range(2):
                tin = pool.tile([gr[gi], gw[gi]], f16, name=f"pm{gi}",
                                tag=f"pm{gi}")
                eng = nc.sync if gi == 0 else nc.scalar
                eng.dma_start(out=tin[:], in_=ins[gi][:])
                gtiles.append(tin)
            # chunk i -> (group tile, column offset within the group)
            views = {}
            for gi, g in enumerate(groups):
                o = 0
                for i in g:
                    cw = layout[i][0]
                    views[i] = (gtiles[gi], o)
                    o += 128 + cw
            # store queues: first on HWDGE for a prompt drain start, the
            # third on SWDGE (its ~1.2us doorbell latency hides behind
            # the already-draining earlier stores), last on HWDGE (sync
            # is free again) so the final completion isn't SWDGE-delayed.
            store_eng = [nc.sync, nc.scalar, nc.gpsimd, nc.sync]
            off = 0
            for i, (cw, R) in enumerate(layout):
                tin, o = views[i]
                ps = psum_pool.tile([P, cw], f32, name=f"ps{i}", tag=f"ps{i}")
                nc.tensor.matmul(ps[:], tin[:, o:o + 128],
                                 tin[:, o + 128:o + 128 + cw])
                ot = pool.tile([P, cw], f32, name=f"o{i}", tag=f"o{i}")
                if i % 2 == 0:
                    nc.vector.tensor_copy(out=ot[:], in_=ps[:])
                else:
                    nc.scalar.copy(out=ot[:], in_=ps[:])
                store_eng[i % 4].dma_start(out=y_dram[:, off:off + cw],
                                           in_=ot[:])
                off += cw

    nc.compile()
    return nc


def _get_compiled(layout):
    if layout not in _compiled:
        _compiled[layout] = _build(layout)
    return _compiled[layout]


def _in_maps(arrays, layout):
    h = (len(layout) + 1) // 2
    return [{"pm0": np.ascontiguousarray(np.concatenate(arrays[c][:h], axis=1)),
             "pm1": np.ascontiguousarray(np.concatenate(arrays[c][h:], axis=1))}
            for c in range(NCORES)]


def kernel(segment_x, segment_y):
    from concourse.bass_utils import run_bass_kernel_spmd

    arrays, layout = _host_prep(segment_x, segment_y)
    nc = _get_compiled(layout)
    in_maps = _in_maps(arrays, layout)
    res = run_bass_kernel_spmd(nc, in_maps, core_ids=list(range(NCORES)))

    out = np.empty((B, S), dtype=np.float32)
    for c in range(NCORES):
        yc = res.results[c]["y"]  # [128, 1536]
        base = c * PIX_PER_CORE
        out[:, base:base + PIX_PER_CORE] = yc.reshape(B, RPB * W)
    return out



# revision 2
# speedup vs baseline: 2.0080x; 2.0080x over previous
"""Trainium2 Bass kernel for nn_Decoder_58514634440787 (histogram_binning).

Piecewise-linear decoder: y[b, s] = interp of (segment_x, segment_y) knots
evaluated at the uniform pixel grid t_s = (s+1)/S, S = 196608, B = 8.

Pixels are sharded across 8 cores (24576 each), laid out on-chip as
[128 partitions = 8 batches x 16 rows, 1536 pixels].  The host builds a
tiny fp16 piecewise-linear *basis* per column chunk so the whole pixel
computation is one single-pass fp16 matmul per chunk on the TensorEngine
(see _host_prep).  Device program is raw Bass (no TileContext):

  - 2 input DMAs (HWDGE; the first-needed group on the Scalar queue,
    whose NEFF preamble finishes earliest)
  - per chunk: PE matmul -> PSUM; DVE/ACT cast PSUM f32 -> SBUF fp16;
    store DMA fp16 -> DRAM over the SWDGE + both HWDGE queues
  - hand-rolled semaphores, one shared store-completion semaphore,
    single SP wait at the end
  - the framework's const-pool memsets + entry all-engine barrier are
    stripped (every cross-engine dep here is explicitly sem-guarded)

fp16 output staging costs ~2e-4 norm rel error (gate is 2e-2); the host
upconverts to f32.  Inputs are the full [8, 33] knot tensors;
sharding/gather happens in kernel().
"""

import numpy as np

S = 196608
B = 8
W = 1536              # pixels per partition row
RPB = 16              # rows per batch per core
P = 128               # partitions = B * RPB
NCORES = 8
PIX_PER_CORE = RPB * W  # 24576

_t_grid = None          # f32 [S] exact (s+1)/S
_compiled = {}          # layout tuple -> nc

_R_BUCKETS = (8, 12, 16, 24, 32, 48, 64, 96, 128)
_BASE_WIDTHS = (256, 512, 512, 256)


def _get_grid():
    global _t_grid
    if _t_grid is None:
        _t_grid = (np.arange(1, S + 1, dtype=np.float64) / S).astype(np.float32)
    return _t_grid


def _fix_x_order(sx, sy):
    """Running max of x along the segment axis, y carried from the position
    achieving the max (ties keep the later entry). Matches reference."""
    x = sx.copy()
    y = sy.copy()
    for b in range(sx.shape[0]):
        cx, cy = sx[b, 0], sy[b, 0]
        for i in range(sx.shape[1]):
            if sx[b, i] >= cx:
                cx, cy = sx[b, i], sy[b, i]
            x[b, i] = cx
            y[b, i] = cy
    return x, y


def _pow2_scale(vmax):
    """Power-of-two s with vmax*s ~ 2^4, so fp16(v*s) avoids subnormals
    and overflow.  The inverse 2^-e (|e| <= 24) is exact in fp16, and so
    is f * 2^-e for integer f <= 2047 (subnormals included)."""
    if vmax <= 0.0 or not np.isfinite(vmax):
        return 1.0
    e = 4 - int(np.floor(np.log2(vmax)))
    e = min(max(e, -10), 24)
    return float(2.0 ** e)


def _host_prep(segment_x, segment_y):
    """Returns (in_maps_arrays, layout).

    layout: tuple of (chunk_width, R) pairs, same for every core.
    in_maps_arrays: [core][chunk] -> fp16 [R, 128 + chunk_width] array
    holding lhsT (cols 0:128) and the moving basis M (cols 128:).
    """
    t_grid = _get_grid()
    sx = np.asarray(segment_x, dtype=np.float32)
    sy = np.asarray(segment_y, dtype=np.float32)
    x, y = _fix_x_order(sx, sy)

    gaps = x[:, 1:] - x[:, :-1]
    div = np.where(gaps == 0.0, np.float32(0.0001), gaps).astype(np.float32)
    a = ((y[:, 1:] - y[:, :-1]) / div).astype(np.float32)          # [B, 32]
    a64 = a.astype(np.float64)
    x64 = x.astype(np.float64)
    y64 = y.astype(np.float64)

    # First pixel index s with t_s >= x_n, for knots n = 1..31.
    # searchsorted on the exact f32 grid == the reference's f32 compares.
    k = np.stack([np.searchsorted(t_grid, x[b, 1:32], side='left')
                  for b in range(B)])                               # [B, 31]
    ks = [np.sort(k[b]) for b in range(B)]

    def seg(b, s):
        # segment index at pixel s = number of transition pixels <= s
        return int(np.searchsorted(ks[b], s, side='right'))

    def line64(b, m, s0):
        # (slope, intercept) in local column space for segment m of batch b,
        # f local to a span starting at global pixel s0, in float64
        aa = a64[b, m]
        return (aa / S, aa * ((s0 + 1) / S - x64[b, m]) + y64[b, m])

    # transitions[b] = sorted unique global pixels where the segment changes
    transitions = [np.unique(k[b][k[b] < S]) for b in range(B)]

    def chunk_transitions(widths):
        """[core][chunk] -> list of (p, k_loc, b, s0) transition entries.
        k_loc in [1, width): a transition at the chunk start is folded
        into the base line."""
        offs = np.concatenate([[0], np.cumsum(widths)]).astype(int)
        out = [[[] for _ in widths] for _ in range(NCORES)]
        for c in range(NCORES):
            for b in range(B):
                for r in range(RPB):
                    row0 = c * PIX_PER_CORE + r * W
                    p = b * RPB + r
                    tr = transitions[b]
                    lo = np.searchsorted(tr, row0, side='right')
                    hi = np.searchsorted(tr, row0 + W - 1, side='right')
                    for g in tr[lo:hi]:
                        col = int(g) - row0          # 1..W-1
                        ci = int(np.searchsorted(offs, col, side='right')) - 1
                        k_loc = col - int(offs[ci])
                        if k_loc == 0:
                            continue  # covered by that chunk's base line
                        out[c][ci].append((p, k_loc, b, row0 + int(offs[ci])))
        return out

    # choose chunk widths so every (core, chunk) fits 2*(2+n)<=128 rows
    widths = list(_BASE_WIDTHS)
    while True:
        per = chunk_transitions(widths)
        worst = [max(len(per[c][i]) for c in range(NCORES))
                 for i in range(len(widths))]
        bad = [i for i, n in enumerate(worst) if 2 * (2 + n) > 128]
        if not bad:
            break
        i = bad[0]
        w = widths[i]
        assert w >= 2, "cannot split further"
        widths = widths[:i] + [w // 2, w - w // 2] + widths[i + 1:]

    rs = []
    for i, n in enumerate(worst):
        need = 2 * (2 + n)
        rb = next(rr for rr in _R_BUCKETS if rr >= need)
        rs.append(rb)
    # chunks are shipped as two packed DMAs (first half / second half of
    # the chunk list); chunks sharing a DMA share the partition-dim R
    h = (len(rs) + 1) // 2
    ra = max(rs[:h]); rb = max(rs[h:])
    rs = [ra] * h + [rb] * (len(rs) - h)
    layout = tuple(zip(widths, rs))

    def hilo(v64):
        """Split float64 array/scalar into (hi, lo) fp16 pair."""
        hi = np.asarray(v64, dtype=np.float16)
        lo = np.asarray(v64 - hi.astype(np.float64), dtype=np.float16)
        return hi, lo

    offs = np.concatenate([[0], np.cumsum(widths)]).astype(int)
    arrays = []
    for c in range(NCORES):
        core_arrays = []
        for i, (cw, R) in enumerate(layout):
            arr = np.zeros((R, 128 + cw), dtype=np.float16)
            f = np.arange(cw, dtype=np.float64)

            # base lines per partition, rebased to this chunk's start
            av = np.zeros(P, dtype=np.float64)
            bv = np.zeros(P, dtype=np.float64)
            for b in range(B):
                for r in range(RPB):
                    p = b * RPB + r
                    s0 = c * PIX_PER_CORE + r * W + int(offs[i])
                    al, be = line64(b, seg(b, s0), s0)
                    av[p] = al
                    bv[p] = be

            # rows 0-1: a_hi/a_lo (in lhsT) paired with scaled iota (in M)
            sa = _pow2_scale(float(np.max(np.abs(av))))
            iota = (f / sa).astype(np.float16)  # f * 2^-e: exact
            ahi, alo = hilo(av * sa)
            arr[0, :128] = ahi
            arr[1, :128] = alo
            arr[0, 128:] = iota
            arr[1, 128:] = iota

            # rows 2-3: b_hi/b_lo paired with a scaled constant row
            sb = _pow2_scale(float(np.max(np.abs(bv))))
            bhi, blo = hilo(bv * sb)
            arr[2, :128] = bhi
            arr[3, :128] = blo
            arr[2, 128:] = np.float16(1.0 / sb)
            arr[3, 128:] = np.float16(1.0 / sb)

            # two rows per transition: D_hi / D_lo, one-hot scaled lhsT
            for j, (p, k_loc, b, s0) in enumerate(per[c][i]):
                m_new = seg(b, s0 + k_loc)
                m_old = seg(b, s0 + k_loc - 1)
                al_n, be_n = line64(b, m_new, s0)
                al_o, be_o = line64(b, m_old, s0)
                d = np.where(f >= k_loc,
                             (al_n - al_o) * f + (be_n - be_o), 0.0)
                sd = _pow2_scale(float(np.max(np.abs(d))))
                dhi, dlo = hilo(d * sd)
                arr[4 + 2 * j, 128:] = dhi
                arr[5 + 2 * j, 128:] = dlo
                arr[4 + 2 * j, p] = np.float16(1.0 / sd)
                arr[5 + 2 * j, p] = np.float16(1.0 / sd)
            core_arrays.append(arr)
        arrays.append(core_arrays)
    return arrays, layout


def _build(layout):
    import concourse.bacc as bacc
    import concourse.mybir as mybir

    f16 = mybir.dt.float16
    f32 = mybir.dt.float32

    nc = bacc.Bacc("TRN2", debug=False, enable_asserts=False,
                   enable_partition_id=False, monotonic_sem_count=0)

    # Strip the framework's const-pool memsets and the entry all-engine
    # barrier: we use no const APs, and every cross-engine dependency in
    # this kernel is explicitly semaphore-guarded; the barrier only delays
    # the first input DMA behind the slowest engine's NEFF preamble.
    blk = nc.main_func.blocks[0]
    drop = [i for i in blk.instructions
            if isinstance(i, (mybir.InstMemset, mybir.InstDrain,
                              mybir.InstEventSemaphore))]
    for i in drop:
        blk.instructions.remove(i)

    n = len(layout)
    h = (n + 1) // 2
    groups = [list(range(0, h)), list(range(h, n))]
    gw = [sum(128 + layout[i][0] for i in g) for g in groups]
    gr = [layout[g[0]][1] for g in groups]
    ins = [nc.dram_tensor(f"pm{gi}", [gr[gi], gw[gi]], f16,
                          kind="ExternalInput").ap()
           for gi in range(2)]
    y_dram = nc.dram_tensor("y", [P, W], f16, kind="ExternalOutput").ap()

    # on-chip tensors (static allocation, no pools)
    gtiles = [nc.alloc_sbuf_tensor(f"pm{gi}_sb", [gr[gi], gw[gi]], f16).ap()
              for gi in range(2)]
    ps = [nc.alloc_psum_tensor(f"ps{i}", [P, cw], f32).ap()
          for i, (cw, _) in enumerate(layout)]
    # fp16 staging: halves both the PSUM->SBUF copy time (DVE runs 2x on
    # 16-bit) and the store DMA payload; costs ~4e-4 norm rel error, far
    # inside the 2e-2 gate (the host upconverts to f32)
    ot = [nc.alloc_sbuf_tensor(f"o{i}", [P, cw], f16).ap()
          for i, (cw, _) in enumerate(layout)]

    # semaphores
    in_sem = [nc.alloc_semaphore(f"in{gi}") for gi in range(2)]
    pe_sem = nc.alloc_semaphore("pe")
    dve_sem = nc.alloc_semaphore("dve")
    pl_sem = nc.alloc_semaphore("pl")
    act_sem = nc.alloc_semaphore("actc")
    st_sem = nc.alloc_semaphore("st")

    # chunk -> (group tile, col offset in group)
    views = {}
    for gi, g in enumerate(groups):
        o = 0
        for i in g:
            cw = layout[i][0]
            views[i] = (gtiles[gi], o)
            o += 128 + cw

    # input DMAs: one per group, on the two HWDGE queues.  Group 0 (the
    # chunks PE consumes first) rides the Scalar queue, whose NEFF
    # preamble finishes ~0.5us before Sync's (Sync's final preamble DRAIN
    # waits out the instruction-fetch DMAs).
    nc.scalar.dma_start(out=gtiles[0][:, :], in_=ins[0][:, :]).then_inc(in_sem[0], 16)
    nc.sync.dma_start(out=gtiles[1][:, :], in_=ins[1][:, :]).then_inc(in_sem[1], 16)

    # engine assignment per chunk (GPSIMD cannot read PSUM, so copies
    # alternate DVE/ACT; stores go SWDGE first -- its 994ns generation
    # absorbs early -- then the two HWDGE queues)
    copy_engines = []
    store_engines = []
    for i in range(n):
        copy_engines.append([nc.vector, nc.scalar][i % 2])
        store_engines.append([nc.gpsimd, nc.sync, nc.sync, nc.scalar][i % 4])

    # PE: matmuls in chunk order; wait for each group's input once
    waited = [False, False]
    for i, (cw, R) in enumerate(layout):
        gi = 0 if i in groups[0] else 1
        if not waited[gi]:
            nc.tensor.wait_ge(in_sem[gi], 16)
            waited[gi] = True
        tin, o = views[i]
        nc.tensor.matmul(ps[i][:, :], tin[:, o:o + 128],
                         tin[:, o + 128:o + 128 + cw]).then_inc(pe_sem)

    # copies + stores
    copy_counts = {}
    copy_sems = {id(nc.vector): dve_sem, id(nc.scalar): act_sem,
                 id(nc.gpsimd): pl_sem}
    n_store = 0
    for i, (cw, R) in enumerate(layout):
        ce = copy_engines[i]
        se = store_engines[i]
        csem = copy_sems[id(ce)]
        ce.wait_ge(pe_sem, i + 1)
        if ce is nc.scalar:
            ce.copy(out=ot[i][:, :], in_=ps[i][:, :]).then_inc(csem)
        else:
            ce.tensor_copy(out=ot[i][:, :], in_=ps[i][:, :]).then_inc(csem)
        copy_counts[id(ce)] = copy_counts.get(id(ce), 0) + 1
        if se is not ce:
            se.wait_ge(csem, copy_counts[id(ce)])
        off = sum(layout[j][0] for j in range(i))
        se.dma_start(out=y_dram[:, off:off + cw],
                     in_=ot[i][:, :]).then_inc(st_sem, 16)
        n_store += 1

    # single completion wait
    nc.sync.wait_ge(st_sem, 16 * n_store)

    nc.compile()
    return nc


def _get_compiled(layout):
    if layout not in _compiled:
        _compiled[layout] = _build(layout)
    return _compiled[layout]


def _in_maps(arrays, layout):
    h = (len(layout) + 1) // 2
    return [{"pm0": np.ascontiguousarray(np.concatenate(arrays[c][:h], axis=1)),
             "pm1": np.ascontiguousarray(np.concatenate(arrays[c][h:], axis=1))}
            for c in range(NCORES)]


def kernel(segment_x, segment_y):
    from concourse.bass_utils import run_bass_kernel_spmd

    arrays, layout = _host_prep(segment_x, segment_y)
    nc = _get_compiled(layout)
    in_maps = _in_maps(arrays, layout)
    res = run_bass_kernel_spmd(nc, in_maps, core_ids=list(range(NCORES)))

    out = np.empty((B, S), dtype=np.float32)
    for c in range(NCORES):
        yc = res.results[c]["y"].astype(np.float32)  # [128, 1536] fp16 -> f32
        base = c * PIX_PER_CORE
        out[:, base:base + PIX_PER_CORE] = yc.reshape(B, RPB * W)
    return out


# revision 3
# speedup vs baseline: 2.1394x; 1.0654x over previous
"""Trainium2 Bass kernel for nn_Decoder_58514634440787 (histogram_binning).

Piecewise-linear decoder: y[b, s] = interp of (segment_x, segment_y) knots
evaluated at the uniform pixel grid t_s = (s+1)/S, S = 196608, B = 8.

Pixels are sharded across 8 cores (24576 each), laid out on-chip as
[128 partitions = 8 batches x 16 rows, 1536 pixels].  The host builds a
tiny fp16 piecewise-linear *basis* per column chunk so the whole pixel
computation is one single-pass fp16 matmul per chunk on the TensorEngine
(see _host_prep).  Device program is raw Bass (no TileContext):

  - 2 input DMAs (HWDGE; the first-needed group on the Scalar queue,
    whose NEFF preamble finishes earliest)
  - per chunk: PE matmul -> PSUM; DVE/ACT cast PSUM f32 -> SBUF fp16;
    store DMA fp16 -> DRAM over the SWDGE + both HWDGE queues
  - hand-rolled semaphores, one shared store-completion semaphore,
    single SP wait at the end
  - the framework's const-pool memsets + entry all-engine barrier are
    stripped (every cross-engine dep here is explicitly sem-guarded)

fp16 output staging costs ~2e-4 norm rel error (gate is 2e-2); the host
upconverts to f32.  Inputs are the full [8, 33] knot tensors;
sharding/gather happens in kernel().
"""

import numpy as np

S = 196608
B = 8
W = 1536              # pixels per partition row
RPB = 16              # rows per batch per core
P = 128               # partitions = B * RPB
NCORES = 8
PIX_PER_CORE = RPB * W  # 24576

_t_grid = None          # f32 [S] exact (s+1)/S
_compiled = {}          # layout tuple -> nc

_R_BUCKETS = (8, 12, 16, 24, 32, 48, 64, 96, 128)
_BASE_WIDTHS = (256, 512, 512, 256)


def _get_grid():
    global _t_grid
    if _t_grid is None:
        _t_grid = (np.arange(1, S + 1, dtype=np.float64) / S).astype(np.float32)
    return _t_grid


def _fix_x_order(sx, sy):
    """Running max of x along the segment axis, y carried from the position
    achieving the max (ties keep the later entry). Matches reference."""
    x = sx.copy()
    y = sy.copy()
    for b in range(sx.shape[0]):
        cx, cy = sx[b, 0], sy[b, 0]
        for i in range(sx.shape[1]):
            if sx[b, i] >= cx:
                cx, cy = sx[b, i], sy[b, i]
            x[b, i] = cx
            y[b, i] = cy
    return x, y


def _pow2_scale(vmax):
    """Power-of-two s with vmax*s ~ 2^4, so fp16(v*s) avoids subnormals
    and overflow.  The inverse 2^-e (|e| <= 24) is exact in fp16, and so
    is f * 2^-e for integer f <= 2047 (subnormals included)."""
    if vmax <= 0.0 or not np.isfinite(vmax):
        return 1.0
    e = 4 - int(np.floor(np.log2(vmax)))
    e = min(max(e, -10), 24)
    return float(2.0 ** e)


def _host_prep(segment_x, segment_y):
    """Returns (in_maps_arrays, layout).

    layout: tuple of (chunk_width, R) pairs, same for every core.
    in_maps_arrays: [core][chunk] -> fp16 [R, 128 + chunk_width] array
    holding lhsT (cols 0:128) and the moving basis M (cols 128:).
    """
    t_grid = _get_grid()
    sx = np.asarray(segment_x, dtype=np.float32)
    sy = np.asarray(segment_y, dtype=np.float32)
    x, y = _fix_x_order(sx, sy)

    gaps = x[:, 1:] - x[:, :-1]
    div = np.where(gaps == 0.0, np.float32(0.0001), gaps).astype(np.float32)
    a = ((y[:, 1:] - y[:, :-1]) / div).astype(np.float32)          # [B, 32]
    a64 = a.astype(np.float64)
    x64 = x.astype(np.float64)
    y64 = y.astype(np.float64)

    # First pixel index s with t_s >= x_n, for knots n = 1..31.
    # searchsorted on the exact f32 grid == the reference's f32 compares.
    k = np.stack([np.searchsorted(t_grid, x[b, 1:32], side='left')
                  for b in range(B)])                               # [B, 31]
    ks = [np.sort(k[b]) for b in range(B)]

    def seg(b, s):
        # segment index at pixel s = number of transition pixels <= s
        return int(np.searchsorted(ks[b], s, side='right'))

    def line64(b, m, s0):
        # (slope, intercept) in local column space for segment m of batch b,
        # f local to a span starting at global pixel s0, in float64
        aa = a64[b, m]
        return (aa / S, aa * ((s0 + 1) / S - x64[b, m]) + y64[b, m])

    # transitions[b] = sorted unique global pixels where the segment changes
    transitions = [np.unique(k[b][k[b] < S]) for b in range(B)]

    def chunk_transitions(widths):
        """[core][chunk] -> list of (p, k_loc, b, s0) transition entries.
        k_loc in [1, width): a transition at the chunk start is folded
        into the base line."""
        offs = np.concatenate([[0], np.cumsum(widths)]).astype(int)
        out = [[[] for _ in widths] for _ in range(NCORES)]
        for c in range(NCORES):
            for b in range(B):
                for r in range(RPB):
                    row0 = c * PIX_PER_CORE + r * W
                    p = b * RPB + r
                    tr = transitions[b]
                    lo = np.searchsorted(tr, row0, side='right')
                    hi = np.searchsorted(tr, row0 + W - 1, side='right')
                    for g in tr[lo:hi]:
                        col = int(g) - row0          # 1..W-1
                        ci = int(np.searchsorted(offs, col, side='right')) - 1
                        k_loc = col - int(offs[ci])
                        if k_loc == 0:
                            continue  # covered by that chunk's base line
                        out[c][ci].append((p, k_loc, b, row0 + int(offs[ci])))
        return out

    # choose chunk widths so every (core, chunk) fits 2*(2+n)<=128 rows
    widths = list(_BASE_WIDTHS)
    while True:
        per = chunk_transitions(widths)
        worst = [max(len(per[c][i]) for c in range(NCORES))
                 for i in range(len(widths))]
        bad = [i for i, n in enumerate(worst) if 2 * (2 + n) > 128]
        if not bad:
            break
        i = bad[0]
        w = widths[i]
        assert w >= 2, "cannot split further"
        widths = widths[:i] + [w // 2, w - w // 2] + widths[i + 1:]

    rs = []
    for i, n in enumerate(worst):
        need = 2 * (2 + n)
        rb = next(rr for rr in _R_BUCKETS if rr >= need)
        rs.append(rb)
    # chunks are shipped as two packed DMAs (first half / second half of
    # the chunk list); chunks sharing a DMA share the partition-dim R
    h = (len(rs) + 1) // 2
    ra = max(rs[:h]); rb = max(rs[h:])
    rs = [ra] * h + [rb] * (len(rs) - h)
    layout = tuple(zip(widths, rs))

    def hilo(v64):
        """Split float64 array/scalar into (hi, lo) fp16 pair."""
        hi = np.asarray(v64, dtype=np.float16)
        lo = np.asarray(v64 - hi.astype(np.float64), dtype=np.float16)
        return hi, lo

    offs = np.concatenate([[0], np.cumsum(widths)]).astype(int)
    arrays = []
    for c in range(NCORES):
        core_arrays = []
        for i, (cw, R) in enumerate(layout):
            arr = np.zeros((R, 128 + cw), dtype=np.float16)
            f = np.arange(cw, dtype=np.float64)

            # base lines per partition, rebased to this chunk's start
            av = np.zeros(P, dtype=np.float64)
            bv = np.zeros(P, dtype=np.float64)
            for b in range(B):
                for r in range(RPB):
                    p = b * RPB + r
                    s0 = c * PIX_PER_CORE + r * W + int(offs[i])
                    al, be = line64(b, seg(b, s0), s0)
                    av[p] = al
                    bv[p] = be

            # rows 0-1: a_hi/a_lo (in lhsT) paired with scaled iota (in M)
            sa = _pow2_scale(float(np.max(np.abs(av))))
            iota = (f / sa).astype(np.float16)  # f * 2^-e: exact
            ahi, alo = hilo(av * sa)
            arr[0, :128] = ahi
            arr[1, :128] = alo
            arr[0, 128:] = iota
            arr[1, 128:] = iota

            # rows 2-3: b_hi/b_lo paired with a scaled constant row
            sb = _pow2_scale(float(np.max(np.abs(bv))))
            bhi, blo = hilo(bv * sb)
            arr[2, :128] = bhi
            arr[3, :128] = blo
            arr[2, 128:] = np.float16(1.0 / sb)
            arr[3, 128:] = np.float16(1.0 / sb)

            # two rows per transition: D_hi / D_lo, one-hot scaled lhsT
            for j, (p, k_loc, b, s0) in enumerate(per[c][i]):
                m_new = seg(b, s0 + k_loc)
                m_old = seg(b, s0 + k_loc - 1)
                al_n, be_n = line64(b, m_new, s0)
                al_o, be_o = line64(b, m_old, s0)
                d = np.where(f >= k_loc,
                             (al_n - al_o) * f + (be_n - be_o), 0.0)
                sd = _pow2_scale(float(np.max(np.abs(d))))
                dhi, dlo = hilo(d * sd)
                arr[4 + 2 * j, 128:] = dhi
                arr[5 + 2 * j, 128:] = dlo
                arr[4 + 2 * j, p] = np.float16(1.0 / sd)
                arr[5 + 2 * j, p] = np.float16(1.0 / sd)
            core_arrays.append(arr)
        arrays.append(core_arrays)
    return arrays, layout


def _build(layout):
    import concourse.bacc as bacc
    import concourse.mybir as mybir

    f16 = mybir.dt.float16
    f32 = mybir.dt.float32

    nc = bacc.Bacc("TRN2", debug=False, enable_asserts=False,
                   enable_partition_id=False, monotonic_sem_count=0)

    # Strip the framework's const-pool memsets and the entry all-engine
    # barrier: we use no const APs, and every cross-engine dependency in
    # this kernel is explicitly semaphore-guarded; the barrier only delays
    # the first input DMA behind the slowest engine's NEFF preamble.
    blk = nc.main_func.blocks[0]
    drop = [i for i in blk.instructions
            if isinstance(i, (mybir.InstMemset, mybir.InstDrain,
                              mybir.InstEventSemaphore))]
    for i in drop:
        blk.instructions.remove(i)

    n = len(layout)
    h = (n + 1) // 2
    groups = [list(range(0, h)), list(range(h, n))]
    gw = [sum(128 + layout[i][0] for i in g) for g in groups]
    gr = [layout[g[0]][1] for g in groups]
    ins = [nc.dram_tensor(f"pm{gi}", [gr[gi], gw[gi]], f16,
                          kind="ExternalInput").ap()
           for gi in range(2)]
    y_dram = nc.dram_tensor("y", [P, W], f16, kind="ExternalOutput").ap()

    # on-chip tensors (static allocation, no pools)
    gtiles = [nc.alloc_sbuf_tensor(f"pm{gi}_sb", [gr[gi], gw[gi]], f16).ap()
              for gi in range(2)]
    ps = [nc.alloc_psum_tensor(f"ps{i}", [P, cw], f32).ap()
          for i, (cw, _) in enumerate(layout)]
    # fp16 staging: halves both the PSUM->SBUF copy time (DVE runs 2x on
    # 16-bit) and the store DMA payload; costs ~4e-4 norm rel error, far
    # inside the 2e-2 gate (the host upconverts to f32)
    ot = [nc.alloc_sbuf_tensor(f"o{i}", [P, cw], f16).ap()
          for i, (cw, _) in enumerate(layout)]

    # semaphores
    in_sem = [nc.alloc_semaphore(f"in{gi}") for gi in range(2)]
    pe_sem = nc.alloc_semaphore("pe")
    dve_sem = nc.alloc_semaphore("dve")
    pl_sem = nc.alloc_semaphore("pl")
    act_sem = nc.alloc_semaphore("actc")
    st_sem = nc.alloc_semaphore("st")

    # chunk -> (group tile, col offset in group)
    views = {}
    for gi, g in enumerate(groups):
        o = 0
        for i in g:
            cw = layout[i][0]
            views[i] = (gtiles[gi], o)
            o += 128 + cw

    # input DMAs: one per group, on the two HWDGE queues.  Group 0 (the
    # chunks PE consumes first) rides the Scalar queue, whose NEFF
    # preamble finishes ~0.5us before Sync's (Sync's final preamble DRAIN
    # waits out the instruction-fetch DMAs).
    nc.scalar.dma_start(out=gtiles[0][:, :], in_=ins[0][:, :]).then_inc(in_sem[0], 16)
    nc.sync.dma_start(out=gtiles[1][:, :], in_=ins[1][:, :]).then_inc(in_sem[1], 16)

    # engine assignment per chunk (GPSIMD cannot read PSUM, so copies
    # alternate DVE/ACT; stores go SWDGE first -- its 994ns generation
    # absorbs early -- then the two HWDGE queues)
    copy_engines = []
    store_engines = []
    for i in range(n):
        copy_engines.append([nc.vector, nc.scalar][i % 2])
        store_engines.append([nc.gpsimd, nc.sync, nc.sync, nc.scalar][i % 4])

    # PE: matmuls in chunk order; wait for each group's input once
    waited = [False, False]
    for i, (cw, R) in enumerate(layout):
        gi = 0 if i in groups[0] else 1
        if not waited[gi]:
            nc.tensor.wait_ge(in_sem[gi], 16)
            waited[gi] = True
        tin, o = views[i]
        nc.tensor.matmul(ps[i][:, :], tin[:, o:o + 128],
                         tin[:, o + 128:o + 128 + cw]).then_inc(pe_sem)

    # copies + stores
    copy_counts = {}
    copy_sems = {id(nc.vector): dve_sem, id(nc.scalar): act_sem,
                 id(nc.gpsimd): pl_sem}
    n_store = 0
    for i, (cw, R) in enumerate(layout):
        ce = copy_engines[i]
        se = store_engines[i]
        csem = copy_sems[id(ce)]
        ce.wait_ge(pe_sem, i + 1)
        if ce is nc.scalar:
            ce.copy(out=ot[i][:, :], in_=ps[i][:, :]).then_inc(csem)
        else:
            ce.tensor_copy(out=ot[i][:, :], in_=ps[i][:, :]).then_inc(csem)
        copy_counts[id(ce)] = copy_counts.get(id(ce), 0) + 1
        if se is not ce:
            se.wait_ge(csem, copy_counts[id(ce)])
        off = sum(layout[j][0] for j in range(i))
        se.dma_start(out=y_dram[:, off:off + cw],
                     in_=ot[i][:, :]).then_inc(st_sem, 16)
        n_store += 1

    # No explicit store-completion wait: the walrus epilogue emits a DRAIN
    # per engine queue, which blocks until that engine's outstanding DMAs
    # (including DGE ring entries) finish, so NEFF completion still covers
    # the stores.  (Experimental - validated against the reference.)

    nc.compile()
    return nc


def _get_compiled(layout):
    if layout not in _compiled:
        _compiled[layout] = _build(layout)
    return _compiled[layout]


def _in_maps(arrays, layout):
    h = (len(layout) + 1) // 2
    return [{"pm0": np.ascontiguousarray(np.concatenate(arrays[c][:h], axis=1)),
             "pm1": np.ascontiguousarray(np.concatenate(arrays[c][h:], axis=1))}
            for c in range(NCORES)]


def kernel(segment_x, segment_y):
    from concourse.bass_utils import run_bass_kernel_spmd

    arrays, layout = _host_prep(segment_x, segment_y)
    nc = _get_compiled(layout)
    in_maps = _in_maps(arrays, layout)
    res = run_bass_kernel_spmd(nc, in_maps, core_ids=list(range(NCORES)))

    out = np.empty((B, S), dtype=np.float32)
    for c in range(NCORES):
        yc = res.results[c]["y"].astype(np.float32)  # [128, 1536] fp16 -> f32
        base = c * PIX_PER_CORE
        out[:, base:base + PIX_PER_CORE] = yc.reshape(B, RPB * W)
    return out
